# revision 21
# baseline (speedup 1.0000x reference)
"""Trainium2 Bass kernel for an AttnBlock++ (GroupNorm -> QKV 1x1 conv ->
full softmax attention over HW tokens -> output projection -> residual/sqrt(2)).

Sharding: data-parallel over batch B=8 across the 8 NeuronCores; attention is
fully independent per batch element, so each core processes one [C, H*W]
feature map with no collectives.

Per-core algorithm (C=256, N=H*W=4096, 32 groups):
  - GroupNorm is folded into the QKV weights: hn = fs*x + fb (per-channel
    affine from group stats), so q/k/v = (W*diag(fs)) @ x + const. The K-side
    constant drops out of softmax exactly (shift invariance); the V-side
    constant is folded into the output-projection bias (attention rows sum
    to 1); only the Q-side constant is applied.
  - Attention runs in fp8 with DoubleRow matmuls (contraction 256 per
    instruction): q/k are written as fp8e4 in a [128, 2, N] layout (channel
    chunks packed in the middle dim), scores are one DoubleRow matmul per
    128-key tile, exp outputs go straight to fp8e5 (max score*scale ~ 9.6 ->
    e^9.6 ~ 15k < 57344 = e5m2 max, so no shift is needed; quantization
    noise averages out across 4096 keys), and att@V / the softmax
    denominator are DoubleRow matmuls over 256-key pairs.
  - exp is batched over [128, 1024] PSUM regions (two score tiles) to
    amortize the scalar engine's per-instruction overhead.
  - Scores are computed transposed (S_T[m, n], keys on partitions) so no
    transposes are ever needed; the denominator uses an all-ones stationary
    (broadcast across partitions) and division by it commutes with the
    output projection, applied at the very end.
  - 1/sqrt(2) of the residual is folded into the output weights and biases.
"""

import math

import numpy as np

import concourse.bacc as bacc
import concourse.tile as tile
from concourse.tile import add_dep_helper
from concourse import mybir
from concourse import bass_utils

B, C, H, W = 8, 256, 64, 64
N = H * W  # 4096
G = 32  # groups
GD = C // G  # 8 channels per group
EPS = 1e-6
NCORES = 8
NCH = 2  # channel chunks of 128
NBLK = 8  # query blocks
BLK = 512  # queries per block
MT = 32  # key tiles of 128
NKP = MT // 2  # key pairs of 256 (DoubleRow contraction)
SCALE = float(C) ** -0.5  # 1/16
INV_SQRT2 = float(1.0 / math.sqrt(2.0))
NWARM = 40  # HAM warm-up matmuls issued while phase 0 runs

FP32 = mybir.dt.float32
BF16 = mybir.dt.bfloat16
F8E4 = mybir.dt.float8e4
F8E5 = mybir.dt.float8e5
AF = mybir.ActivationFunctionType
ALU = mybir.AluOpType
DR = mybir.MatmulPerfMode.DoubleRow


def build_program():
    nc = bacc.Bacc("TRN2", target_bir_lowering=False, debug=False)

    x = nc.dram_tensor("x", [C, N], FP32, kind="ExternalInput").ap()
    wqT = nc.dram_tensor("wqT", [C, C], FP32, kind="ExternalInput").ap()
    wkT = nc.dram_tensor("wkT", [C, C], FP32, kind="ExternalInput").ap()
    wvT = nc.dram_tensor("wvT", [C, C], FP32, kind="ExternalInput").ap()
    woT = nc.dram_tensor("woT", [C, C], FP32, kind="ExternalInput").ap()
    # wvoT[c_in, c_out] = (w_v.T @ w_o.T)/sqrt(2): output projection folded
    # into the V weights on the host (fs scaling still applied on device)
    wvoT = nc.dram_tensor("wvoT", [C, C], FP32, kind="ExternalInput").ap()
    bq = nc.dram_tensor("bq", [C, 1], FP32, kind="ExternalInput").ap()
    bv = nc.dram_tensor("bv", [C, 1], FP32, kind="ExternalInput").ap()
    bo = nc.dram_tensor("bo", [C, 1], FP32, kind="ExternalInput").ap()
    gns = nc.dram_tensor("gns", [C, 1], FP32, kind="ExternalInput").ap()
    gnb = nc.dram_tensor("gnb", [C, 1], FP32, kind="ExternalInput").ap()
    # ind16[c, g] = 1/8 if c//8 == g else 0 (group-average over channels)
    ind16 = nc.dram_tensor("ind16", [128, 16], FP32, kind="ExternalInput").ap()
    # bcast16[g, c] = 1 if c//8 == g else 0 (broadcast group value to channels)
    bcast16 = nc.dram_tensor("bcast16", [16, 128], FP32, kind="ExternalInput").ap()
    y = nc.dram_tensor("y", [C, N], FP32, kind="ExternalOutput").ap()

    with tile.TileContext(nc) as tc:
        with (
            tc.tile_pool(name="persist", bufs=1) as P,
            tc.tile_pool(name="work", bufs=2) as WK,
        ):
            # ---------------- constants + HAM warm-up ----------------
            junk_bf = P.tile([128, BLK], BF16, tag="junk")
            nc.gpsimd.memset(junk_bf, 0.0)
            ones_dr = P.tile([128, 2, 128], F8E5, tag="ones_dr")
            nc.vector.memset(ones_dr, 1.0)
            ones_b1 = P.tile([128, 1], BF16, tag="ones_b1")
            nc.vector.memset(ones_b1, 1.0)
            eps16 = P.tile([16, 1], FP32, tag="eps16")
            nc.vector.memset(eps16, EPS)
            # dummy Sqrt: preloads the scalar-engine activation table during
            # the DMA window instead of on the phase-0 critical path
            tbl_warm = P.tile([16, 1], FP32, tag="tblwarm")
            nc.scalar.activation(out=tbl_warm, in_=eps16, func=AF.Sqrt)

            # ---------------- load inputs (x first: stats critical path) ----
            x_sb = []
            for j in range(NCH):
                t = P.tile([128, N], FP32, tag=f"x{j}", name=f"x{j}")
                x_sb.append(t)
            xdmas = []
            for p in range(8):  # interleave chunks piece-wise across many
                for j in (1, 0):  # queues; both stats paths pipeline with DMA
                    cs = slice(p * 512, (p + 1) * 512)
                    xdmas.append(nc.sync.dma_start(
                        out=x_sb[j][:, cs], in_=x[j * 128:(j + 1) * 128, cs]
                    ))

            def load2(ap, name, width):
                ts = []
                for j in range(NCH):
                    t = P.tile([128, width], FP32, tag=f"{name}{j}", name=f"{name}{j}")
                    nc.sync.dma_start(out=t, in_=ap[j * 128:(j + 1) * 128, :])
                    ts.append(t)
                return ts

            ind16_sb = P.tile([128, 16], FP32, tag="ind16")
            nc.sync.dma_start(out=ind16_sb, in_=ind16)
            bcast16_sb = P.tile([16, 128], FP32, tag="bcast16")
            nc.sync.dma_start(out=bcast16_sb, in_=bcast16)
            gns_sb = load2(gns, "gns", 1)
            gnb_sb = load2(gnb, "gnb", 1)
            bq_sb = load2(bq, "bq", 1)
            bv_sb = load2(bv, "bv", 1)
            bo_sb = load2(bo, "bo", 1)
            wqT_sb = load2(wqT, "wqT", C)
            wkT_sb = load2(wkT, "wkT", C)
            wvT_sb = load2(wvT, "wvT", C)
            woT_sb = load2(woT, "woT", C)
            wvoT_sb = load2(wvoT, "wvoT", C)

            with tc.tile_pool(name="psum_s", bufs=1, space="PSUM") as PSS, \
                 tc.tile_pool(name="psum_av", bufs=1, space="PSUM") as PSAV, \
                 tc.tile_pool(name="psum_d", bufs=1, space="PSUM") as PSD:
                # One psum layout for the whole kernel (8 banks):
                #   s2[0..2] (2 banks each: 3-deep score ring) | av | d
                # Phase 0/1 matmuls reuse the same banks via slices.
                s2 = [
                    PSS.tile([128, 2 * BLK], FP32, tag=f"s2_{r}", name=f"s2_{r}")
                    for r in range(3)
                ]
                av_slot = PSAV.tile([128, BLK], FP32, tag="av", name="av_slot")

                # HAM warm-up: keep the PE busy while stats/DMA run so the
                # projections and attention run at 2.4 GHz from the start;
                # the trickle chained to the DMA pieces spreads junk matmuls
                # across the whole load window (PE would otherwise idle
                # >3.4us and re-throttle to 1.2 GHz).
                for _ in range(NWARM):
                    nc.tensor.matmul(
                        av_slot, junk_bf[:, 0:128], junk_bf, start=True, stop=True
                    )
                for dins in xdmas:
                    for _ in range(3):
                        jm = nc.tensor.matmul(
                            av_slot[:, 0:128], junk_bf[:, 0:128],
                            junk_bf[:, 0:128], start=True, stop=True,
                        )
                        add_dep_helper(jm.ins, dins.ins, sync=True,
                                       reason="HAM trickle on DMA")

                # ---------------- phase 0: group stats -> folded affine ------
                # chunk 0 stats on the vector engine (bn_stats),
                # chunk 1 stats on the scalar engine (accum of x and x^2);
                # x is cast to the fp8e4 DoubleRow layout on the way.
                x_f8 = P.tile([128, NCH, N], F8E4, tag="xf8")
                for p in range(4):
                    ps = slice(p * 1024, (p + 1) * 1024)
                    nc.vector.tensor_copy(
                        out=x_f8[:, 0, ps], in_=x_sb[0][:, ps]
                    )

                t2 = []  # per chunk [128, 2]: col0 = mean, col1 = E[x^2]
                # chunk 0: bn_stats path
                stats = WK.tile([128, 8, 6], FP32, tag="bnstats")
                for s in range(8):
                    nc.vector.bn_stats(
                        out=stats[:, s, :], in_=x_sb[0][:, s * 512:(s + 1) * 512]
                    )
                mv = WK.tile([128, 2], FP32, tag="bnmv")
                nc.vector.bn_aggr(out=mv, in_=stats)
                t2_0 = WK.tile([128, 2], FP32, tag="chstat0")
                nc.vector.tensor_copy(out=t2_0[:, 0:1], in_=mv[:, 0:1])
                sq = WK.tile([128, 1], FP32, tag="chsq")
                nc.vector.tensor_mul(out=sq, in0=mv[:, 0:1], in1=mv[:, 0:1])
                nc.vector.tensor_add(out=t2_0[:, 1:2], in0=mv[:, 1:2], in1=sq)
                t2.append(t2_0)
                # chunk 1: scalar-engine accumulation path (also makes
                # xb16[1]); split into 4 pieces so it pipelines with the DMA.
                sq_scr = P.tile([128, N], BF16, tag="sqscr")
                xsum_p = WK.tile([128, 8], FP32, tag="xsump")
                for p in range(4):
                    ps = slice(p * 1024, (p + 1) * 1024)
                    nc.scalar.activation(
                        out=x_f8[:, 1, ps], in_=x_sb[1][:, ps], func=AF.Copy,
                        accum_out=xsum_p[:, p:p + 1],
                    )
                    nc.scalar.activation(
                        out=sq_scr[:, ps], in_=x_sb[1][:, ps], func=AF.Square,
                        accum_out=xsum_p[:, 4 + p:5 + p],
                    )
                t2_1 = WK.tile([128, 2], FP32, tag="chstat1")
                sab = WK.tile([128, 4], FP32, tag="sab")
                nc.vector.tensor_add(
                    out=sab[:, 0:2], in0=xsum_p[:, 0:2], in1=xsum_p[:, 2:4]
                )
                nc.vector.tensor_add(
                    out=sab[:, 2:4], in0=xsum_p[:, 4:6], in1=xsum_p[:, 6:8]
                )
                nc.vector.tensor_add(
                    out=t2_1[:, 0:1], in0=sab[:, 0:1], in1=sab[:, 1:2]
                )
                nc.vector.tensor_add(
                    out=t2_1[:, 1:2], in0=sab[:, 2:3], in1=sab[:, 3:4]
                )
                nc.vector.tensor_scalar_mul(out=t2_1, in0=t2_1, scalar1=1.0 / N)
                t2.append(t2_1)

                gmr = []  # [16, 2] per chunk: col0 = group mean, col1 = rstd
                for j in range(NCH):
                    ps_g = s2[j][0:16, 0:2]
                    nc.tensor.matmul(ps_g, ind16_sb, t2[j], start=True, stop=True)
                    g2 = WK.tile([16, 2], FP32, tag="gstat")
                    nc.vector.tensor_copy(out=g2, in_=ps_g)
                    gsq = WK.tile([16, 1], FP32, tag="gsq")
                    nc.vector.tensor_mul(out=gsq, in0=g2[:, 0:1], in1=g2[:, 0:1])
                    gvar = WK.tile([16, 1], FP32, tag="gvar")
                    nc.vector.tensor_sub(out=gvar, in0=g2[:, 1:2], in1=gsq)
                    gsd = WK.tile([16, 1], FP32, tag="gsd")
                    nc.scalar.activation(
                        out=gsd, in_=gvar, func=AF.Sqrt, bias=eps16, scale=1.0
                    )
                    gm_r = WK.tile([16, 2], FP32, tag=f"gmr{j}")
                    nc.vector.tensor_copy(out=gm_r[:, 0:1], in_=g2[:, 0:1])
                    nc.vector.reciprocal(out=gm_r[:, 1:2], in_=gsd)
                    gmr.append(gm_r)
                # dummy Exp: switches the activation table set off the
                # prologue critical path (the set load costs ~2.7us)
                if len(gmr) == NCH:
                    nc.scalar.activation(out=tbl_warm, in_=eps16, func=AF.Exp)

                fs_sb, fb_sb = [], []
                for j in range(NCH):
                    ps_bc = s2[j][:, BLK:BLK + 2]
                    nc.tensor.matmul(ps_bc, bcast16_sb, gmr[j], start=True, stop=True)
                    mbrb = WK.tile([128, 2], FP32, tag="mbrb")
                    nc.vector.tensor_copy(out=mbrb, in_=ps_bc)
                    fs = P.tile([128, 1], FP32, tag=f"fs{j}", name=f"fs{j}")
                    nc.vector.tensor_mul(out=fs, in0=gns_sb[j], in1=mbrb[:, 1:2])
                    tmp = WK.tile([128, 1], FP32, tag="fbt")
                    nc.vector.tensor_mul(out=tmp, in0=mbrb[:, 0:1], in1=fs)
                    fb = P.tile([128, 1], FP32, tag=f"fb{j}", name=f"fb{j}")
                    nc.vector.tensor_sub(out=fb, in0=gnb_sb[j], in1=tmp)
                    fs_sb.append(fs)
                    fb_sb.append(fb)

                # fp8e4 DoubleRow weights: w'[c_in, c_out] = wT*fs[c_in]*16
                # (x16 keeps typical weight magnitudes out of the e4m3
                # subnormal range; the 1/16 is folded into the psum drains)
                wq_dr = P.tile([128, NCH, C], F8E4, tag="wqdr")
                wk_dr = P.tile([128, NCH, C], F8E4, tag="wkdr")
                wvo_dr = P.tile([128, NCH, C], F8E4, tag="wvodr")
                for wdst, wsrc in ((wq_dr, wqT_sb), (wk_dr, wkT_sb),
                                   (wvo_dr, wvoT_sb)):
                    for j in range(NCH):
                        nc.vector.tensor_scalar(
                            out=wdst[:, j, :], in0=wsrc[j],
                            scalar1=fs_sb[j], scalar2=16.0,
                            op0=ALU.mult, op1=ALU.mult,
                        )

                # cQ = Wq @ fb + bq ; cV = Wv @ fb + bv ; bo_eff = Wo @ cV + bo
                cq_sb, cv_sb, boe_sb = [], [], []
                rot = [0]
                tiny_slots = [av_slot[:, 300:301], av_slot[:, 304:305],
                              av_slot[:, 308:309], av_slot[:, 312:313]]

                def tiny_mm(wT, rhs2):
                    ps_c = tiny_slots[rot[0] % 4]
                    rot[0] += 1
                    nc.tensor.matmul(
                        ps_c, wT[0][:, :], rhs2[0], start=True, stop=False,
                    )
                    nc.tensor.matmul(
                        ps_c, wT[1][:, :], rhs2[1], start=False, stop=True,
                    )
                    return ps_c

                # only cQ is on the critical path (the q-projection drains
                # add it); cV/bo_eff gate nothing until the first epoch's y
                # assembly, so they are emitted after the prologue.
                for o in range(NCH):
                    ps_c = tiny_mm(
                        [wqT_sb[0][:, o * 128:(o + 1) * 128],
                         wqT_sb[1][:, o * 128:(o + 1) * 128]],
                        fb_sb,
                    )
                    t = P.tile([128, 1], FP32, tag=f"cq{o}", name=f"cq{o}")
                    nc.vector.tensor_add(out=t, in0=ps_c, in1=bq_sb[o])
                    cq_sb.append(t)

                def emit_cv_boe():
                    for o in range(NCH):
                        ps_c = tiny_mm(
                            [wvT_sb[0][:, o * 128:(o + 1) * 128],
                             wvT_sb[1][:, o * 128:(o + 1) * 128]],
                            fb_sb,
                        )
                        t = P.tile([128, 1], FP32, tag=f"cv{o}", name=f"cv{o}")
                        nc.vector.tensor_add(out=t, in0=ps_c, in1=bv_sb[o])
                        cv_sb.append(t)
                    for o in range(NCH):
                        ps_c = tiny_mm(
                            [woT_sb[0][:, o * 128:(o + 1) * 128],
                             woT_sb[1][:, o * 128:(o + 1) * 128]],
                            cv_sb,
                        )
                        t = P.tile([128, 1], FP32, tag=f"boe{o}", name=f"boe{o}")
                        nc.vector.tensor_add(out=t, in0=ps_c, in1=bo_sb[o])
                        boe_sb.append(t)

                # ---------------- phase 1: Q/K projections ----------------
                # q/k live in fp8e4 DoubleRow layout: [128, chunk(2), N]
                q_f8 = P.tile([128, NCH, N], F8E4, tag="qf8")
                k_f8 = P.tile([128, NCH, N], F8E4, tag="kf8")
                vt_f8 = P.tile([128, MT, C], F8E4, tag="vt")
                e_buf = [
                    P.tile([128, MT, BLK], F8E5, tag=f"ebuf{p}", name=f"ebuf{p}")
                    for p in range(2)
                ]
                e_flat = [t.rearrange("p a b -> p (a b)") for t in e_buf]

                pcnt = [0]

                def proj_qk(which, o, nb):
                    cs = slice(nb * BLK, (nb + 1) * BLK)
                    hs = pcnt[0] % 6
                    ps = s2[hs // 2][:, (hs % 2) * BLK:(hs % 2 + 1) * BLK]
                    pcnt[0] += 1
                    wdr = wq_dr if which == "q" else wk_dr
                    nc.tensor.matmul(
                        ps, wdr[:, :, o * 128:(o + 1) * 128], x_f8[:, :, cs],
                        start=True, stop=True, perf_mode=DR,
                    )
                    if which == "q":
                        # q = psum/16 + cQ, straight to fp8e4
                        nc.vector.tensor_scalar(
                            out=q_f8[:, o, cs], in0=ps,
                            scalar1=1.0 / 16.0, scalar2=cq_sb[o],
                            op0=ALU.mult, op1=ALU.add,
                        )
                    else:
                        nc.vector.tensor_scalar_mul(
                            out=k_f8[:, o, cs], in0=ps, scalar1=1.0 / 16.0
                        )

                # K for all blocks first (scores need every key), then Q of
                # block 0 only - block 0's score pass starts right after, and
                # the remaining Q blocks are projected underneath it.
                for o in range(NCH):
                    for nb in range(NBLK):
                        proj_qk("k", o, nb)
                for o in range(NCH):
                    proj_qk("q", o, 0)

                # ---------------- phase 2: attention ----------------
                # Software-pipelined by one full query block: epoch j computes
                # S+exp for block j while the tensor engine consumes block
                # j-1's exp results (d/av matmuls). The consumers read data
                # a full epoch old, so their waits are essentially always
                # satisfied; the out-of-order Tile scheduler zips producers
                # (gated by the 3-deep score-ring/exp round trip) into the
                # consumer stream and across epoch boundaries.
                def emit_score(nb, k):
                    # one DoubleRow matmul per 128-key tile (contraction 256);
                    # exp drains two tiles at once from a [128, 1024] region.
                    # High priority: the exp stream is the kernel's rate
                    # floor, so score production must never be starved by
                    # leftover consumer matmuls in the scheduler.
                    ms = slice(k * 128, (k + 1) * 128)
                    cs = slice(nb * BLK, (nb + 1) * BLK)
                    reg = s2[((nb * MT + k) // 2) % 3]
                    half = k % 2
                    with tc.high_priority():
                        nc.tensor.matmul(
                            reg[:, half * BLK:(half + 1) * BLK],
                            k_f8[:, :, ms], q_f8[:, :, cs],
                            start=True, stop=True, perf_mode=DR,
                        )
                        if half == 1:
                            nc.scalar.activation(
                                out=e_flat[nb % 2][:, (k - 1) * BLK:(k + 1) * BLK],
                                in_=reg, func=AF.Exp, scale=SCALE,
                            )

                # prologue: V' projections (output-projection pre-folded into
                # the weights) interleaved with block 0's S+exp pass; the v
                # accumulators double-buffer across the av and d banks.
                for k in range(MT):
                    ms = slice(k * 128, (k + 1) * 128)
                    if k % 2 == 0:
                        ps_v = av_slot[:, 0:C]
                    else:
                        ps_v = PSD.tile([128, BLK], FP32, tag="d",
                                        name=f"vslot{k}")[:, 0:C]
                    nc.tensor.matmul(
                        ps_v, x_f8[:, :, ms], wvo_dr, start=True, stop=True,
                        perf_mode=DR,
                    )
                    nc.vector.tensor_scalar_mul(
                        out=vt_f8[:, k, :], in0=ps_v, scalar1=1.0 / 16.0
                    )
                    emit_score(0, k)
                # remaining Q blocks: needed one epoch ahead of their scores
                for o in range(NCH):
                    for nb in range(1, NBLK):
                        proj_qk("q", o, nb)
                emit_cv_boe()

                for j in range(1, NBLK + 1):
                    c = j - 1  # consumer block
                    ccs = slice(c * BLK, (c + 1) * BLK)
                    eb = e_buf[c % 2]

                    # producers first in program order = higher scheduler
                    # priority; the consumer groups below fill PE idle while
                    # the score ring waits on exp.
                    if j < NBLK:
                        for k in range(MT):
                            emit_score(j, k)

                    # Pair-sums of e on the vector engine (fp8e5: pair max
                    # ~27.5k < 57344) start at epoch begin and halve the
                    # denominator matmul count; the d group runs after the av
                    # groups so nothing gates on the DVE cascade. The d matmul
                    # uses a single stationary ones-column (M=1): one psum row
                    # instead of a 128-row broadcast - 1/128th of the MAC
                    # energy - and gpsimd broadcasts the reciprocal.
                    prs = WK.tile([128, NKP, BLK], F8E5, tag="prs")
                    for kp in range(NKP):
                        nc.vector.tensor_add(
                            out=prs[:, kp, :], in0=eb[:, 2 * kp, :],
                            in1=eb[:, 2 * kp + 1, :],
                        )

                    av_sb = []
                    for o in range(NCH):
                        for kp in range(NKP):
                            nc.tensor.matmul(
                                av_slot,
                                vt_f8[:, 2 * kp:2 * kp + 2, o * 128:(o + 1) * 128],
                                eb[:, 2 * kp:2 * kp + 2, :],
                                start=(kp == 0), stop=(kp == NKP - 1),
                                perf_mode=DR,
                            )
                        t = WK.tile([128, BLK], BF16, tag=f"avsb{o}", name=f"avsb{o}")
                        nc.vector.tensor_copy(out=t, in_=av_slot)
                        av_sb.append(t)

                    ps_d = PSD.tile([128, BLK], FP32, tag="d", name=f"d{j}")
                    for qi in range(NKP // 2):
                        nc.tensor.matmul(
                            ps_d[0:1, :], ones_dr[:, :, 0:1],
                            prs[:, 2 * qi:2 * qi + 2, :],
                            start=(qi == 0), stop=(qi == NKP // 2 - 1),
                            perf_mode=DR,
                        )
                    d_sb = WK.tile([1, BLK], FP32, tag="dsb")
                    nc.vector.tensor_copy(out=d_sb, in_=ps_d[0:1, :])
                    rb_1 = WK.tile([1, BLK], FP32, tag="rb1")
                    nc.vector.reciprocal_approx_fast(rb_1, d_sb)
                    rb_sb = WK.tile([128, BLK], FP32, tag="rbsb")
                    nc.gpsimd.partition_broadcast(rb_sb, rb_1)

                    for o in range(NCH):
                        # y = x/sqrt2 + bo_eff/sqrt2 + AV'/denom
                        xb_t = WK.tile([128, BLK], FP32, tag="xbt")
                        nc.vector.tensor_scalar(
                            out=xb_t, in0=x_sb[o][:, ccs],
                            scalar1=boe_sb[o], scalar2=INV_SQRT2,
                            op0=ALU.add, op1=ALU.mult,
                        )
                        t_t = WK.tile([128, BLK], FP32, tag="tt2")
                        nc.vector.tensor_tensor(
                            out=t_t, in0=av_sb[o], in1=rb_sb, op=ALU.mult
                        )
                        y_t = WK.tile([128, BLK], FP32, tag="yt")
                        nc.vector.tensor_add(out=y_t, in0=t_t, in1=xb_t)
                        nc.sync.dma_start(
                            out=y[o * 128:(o + 1) * 128, ccs], in_=y_t
                        )

    nc.compile()
    return nc


_PROGRAM = None


def _get_program():
    global _PROGRAM
    if _PROGRAM is None:
        _PROGRAM = build_program()
    return _PROGRAM


def make_in_maps(inputs):
    x = np.ascontiguousarray(np.asarray(inputs["x"], dtype=np.float32))
    shared = {
        "wqT": np.ascontiguousarray(np.asarray(inputs["w_q"], np.float32).T),
        "wkT": np.ascontiguousarray(np.asarray(inputs["w_k"], np.float32).T),
        "wvT": np.ascontiguousarray(np.asarray(inputs["w_v"], np.float32).T),
        "woT": np.ascontiguousarray(np.asarray(inputs["w_o"], np.float32).T),
        "wvoT": np.ascontiguousarray(
            (np.asarray(inputs["w_v"], np.float32).T
             @ np.asarray(inputs["w_o"], np.float32).T) * INV_SQRT2
        ),
        "bq": np.asarray(inputs["b_q"], np.float32).reshape(C, 1).copy(),
        "bv": np.asarray(inputs["b_v"], np.float32).reshape(C, 1).copy(),
        "bo": np.asarray(inputs["b_o"], np.float32).reshape(C, 1).copy(),
        "gns": np.asarray(inputs["gn_scale"], np.float32).reshape(C, 1).copy(),
        "gnb": np.asarray(inputs["gn_bias"], np.float32).reshape(C, 1).copy(),
        "ind16": (
            (np.arange(128)[:, None] // GD == np.arange(16)[None, :]) / GD
        ).astype(np.float32),
        "bcast16": (
            np.arange(16)[:, None] == np.arange(128)[None, :] // GD
        ).astype(np.float32),
    }
    in_maps = []
    for i in range(NCORES):
        m = dict(shared)
        m["x"] = np.ascontiguousarray(x[i].reshape(C, N))
        in_maps.append(m)
    return in_maps


def run(inputs, trace=False, trace_cores=None):
    nc = _get_program()
    in_maps = make_in_maps(inputs)
    res = bass_utils.run_bass_kernel_spmd(
        nc, in_maps, core_ids=list(range(NCORES)), trace=trace,
        trace_cores=trace_cores,
    )
    out = np.stack(
        [res.results[i]["y"].reshape(C, H, W) for i in range(NCORES)]
    ).astype(np.float32)
    return out, res


def kernel(**inputs) -> np.ndarray:
    out, _ = run(inputs, trace=False)
    return out
